# revision 62
# baseline (speedup 1.0000x reference)
"""2D Gaussian Splatting on 8 Trainium2 NeuronCores.

Strategy: shard pixels across cores (embarrassingly parallel); each core owns
32 image rows = 16 tiles of 32x16 px (512 px each). Host-side 3.5-sigma
bounding-box culling gives <=127 gaussians per tile (measured max 121 for this
input distribution), so the front-to-back transmittance scan per tile is a
single 127x127 triangular matmul.

Per tile (phase-ordered across tiles so the ACT table loads only 3x):
  z+b  = -q/2 + bias       K=22 fp32r matmul (hi/lo split -> fp32 exact;
                           bias = log opac + const coef folded in as rows)
  alpha = exp(z + b)       ACT Exp, batched 2 tiles/op from PSUM
  lom  = ln(1 - alpha)     ACT Ln, batched 4 tiles/op
  logT = tri @ lom         K=127 triangular fp32r matmul (exclusive scan)
  T    = exp(logT)         ACT Exp, batched 2 tiles/op from PSUM
  w    = alpha * T         DVE multiply -> bf16
  img  = colors^T @ w      K=127 bf16 matmul -> [1, 512] at psum row 32*(t%4)
"""

import math
import numpy as np

W = 256
H = 256
TILE_W = 32
TILE_H = 16
F = TILE_W * TILE_H      # 512 pixels per tile
NTX = W // TILE_W        # 8 tile cols
NTY = H // TILE_H        # 16 tile rows
NT = 16                  # tiles per core (2 tile rows x 8 tile cols)
MG = 127                 # max gaussians per tile
NB = 5                   # basis monomials: x'^2, x'y', y'^2, x', y'
KQ = 4 * NB + 2          # hi/lo coef x hi/lo basis pairs + bias hi/lo rows
N_CORES = 8
PAD_BIAS = -60.0         # exp(-60) == 0 for padded gaussian slots
SIGMA_K = 3.5

# fp32r q-matmul const tensor layout (columns): basis | coefs
# DMA'd as 4 column-chunks on parallel queues; chunk 0 = basis + tiles 0-3
C_BASIS = 0
C_COEF = F
QB_W = C_COEF + NT * 128
QB_H = KQ
QB_CH = [(0, F + 4 * 128)] + [
    (F + b * 4 * 128, 4 * 128) for b in range(1, 4)
]


def _round_fp32r(a):
    """Round float32 array to fp32r (RNE, drop low 12 mantissa bits)."""
    b = np.asarray(a, np.float32).view(np.uint32).astype(np.uint64)
    r = (b + 0x7FF + ((b >> 12) & 1)) & 0xFFFFF000
    return r.astype(np.uint32).view(np.float32)


def _split_fp32r(a):
    """Split float32 array into fp32r hi + lo with hi+lo ~= a to ~2^-24."""
    a = np.asarray(a, np.float32)
    hi = _round_fp32r(a)
    lo = _round_fp32r(a - hi)
    return hi, lo


def _build_nc():
    import concourse.bacc as bacc
    import concourse.mybir as mybir
    from concourse.tile import TileContext

    f32 = mybir.dt.float32
    f32r = mybir.dt.float32r
    bf16 = mybir.dt.bfloat16
    AF = mybir.ActivationFunctionType

    nc = bacc.Bacc("TRN2", target_bir_lowering=False, debug=False)
    qb_d = nc.declare_dram_parameter("qb", [QB_H, QB_W], f32r, isOutput=False)
    tri_d = nc.declare_dram_parameter("tri", [MG, MG], f32r, isOutput=False)
    colors_d = nc.declare_dram_parameter("colors", [128, NT], bf16, isOutput=False)
    out_d = nc.declare_dram_parameter("out", [4, 4 * F], f32, isOutput=True)

    with TileContext(nc) as tc:
        with (
            tc.tile_pool(name="const", bufs=1) as cpool,
            tc.tile_pool(name="alpha", bufs=4) as apool,
            tc.tile_pool(name="lom", bufs=4) as lpool,
            tc.tile_pool(name="tt", bufs=3) as tpool,
            tc.tile_pool(name="w", bufs=6) as wpool,
            tc.tile_pool(name="ps", bufs=2, space="PSUM") as pspool,
            tc.tile_pool(name="psi", bufs=1, space="PSUM") as ipool,
        ):
            qbt = cpool.tile([QB_H, QB_W], f32r)
            for off, width in QB_CH:
                nc.sync.dma_start(
                    qbt[:, off : off + width], qb_d[:, off : off + width]
                )
            colorst = cpool.tile([128, NT], bf16)
            nc.sync.dma_start(colorst[:], colors_d[:])
            trit = cpool.tile([MG, MG], f32r)
            nc.sync.dma_start(trit[:], tri_d[:])
            tri = trit[:]
            out_sb = cpool.tile([97, 4 * F], f32)

            # dummy ops: ACT/PE observe the input DMAs (and ACT loads the exp
            # table) on dedicated instructions so real ops carry <=1 wait each
            dummy = cpool.tile([1, 1], f32)
            nc.scalar.activation(dummy[:], colorst[0:1, 0:1], AF.Exp, bias=0.0)

            # persistent img banks: tile t -> bank t//4, partition 32*(t%4);
            # no slot reuse, so no cross-engine WAR waits on any matmul
            psi_banks = [
                ipool.tile([97, F], f32, name=f"psib{b}", tag=f"psi{b}")
                for b in range(4)
            ]
            nc.tensor.matmul(
                psi_banks[0][0:1, 0:1],
                colorst[0:1, 0:1],
                colorst[0:1, 0:1],
                start=True,
                stop=True,
            )
            nc.tensor.matmul(
                psi_banks[0][0:2, 0:126],
                trit[0:1, 0:2],
                trit[0:1, 0:126],
                start=True,
                stop=True,
            )

            # PE pre-warm: ~3.5us of dummy matmuls during the input DMAs so
            # the HAM clock gate opens before the real work starts
            warm = cpool.tile([128, F], bf16)
            nc.gpsimd.memset(warm[:], 0.0)
            for _ in range(10):
                nc.tensor.matmul(
                    psi_banks[1][0:1, :],
                    warm[:, 0:1],
                    warm[:],
                    start=True,
                    stop=True,
                )

            # phase 1 (exp table): z = -q/2 + bias via matmul; alpha = exp(z)
            alpha_grps = [
                apool.tile([MG, 4 * F], f32, name=f"ag{i}", tag="ag")
                for i in range(4)
            ]
            for g in range(8):  # pairs of tiles
                ps = pspool.tile([MG, 2 * F], f32, name="psq")
                for j in range(2):
                    t = 2 * g + j
                    nc.tensor.matmul(
                        ps[:, j * F : (j + 1) * F],
                        qbt[0:KQ, C_COEF + t * 128 : C_COEF + t * 128 + MG],
                        qbt[0:KQ, C_BASIS : C_BASIS + F],
                        start=True,
                        stop=True,
                    )
                nc.scalar.activation(
                    alpha_grps[g // 2][:, (g % 2) * 2 * F : ((g % 2) + 1) * 2 * F],
                    ps[:],
                    AF.Exp,
                    bias=0.0,
                )

            # phase 2 (ln table): lom = ln(1 - alpha), 4 tiles per op
            lom_grps = []
            for i in range(4):
                lom = lpool.tile([MG, 4 * F], f32r, name=f"lg{i}", tag="lg")
                nc.scalar.activation(
                    lom[:], alpha_grps[i][:], AF.Ln, bias=1.0, scale=-1.0
                )
                lom_grps.append(lom)

            # phase 3 (exp table): triangular scan, T = exp(logT),
            # w = alpha * T, img = colors^T @ w
            def consume(g, Tt):
                for j in range(2):
                    t = 2 * g + j
                    w = wpool.tile([MG, F], bf16, name="wt", tag="wt")
                    nc.vector.tensor_mul(
                        w[:],
                        alpha_grps[t // 4][:, (t % 4) * F : (t % 4 + 1) * F],
                        Tt[:, j * F : (j + 1) * F],
                    )
                    b, jj = divmod(t, 4)
                    nc.tensor.matmul(
                        psi_banks[b][32 * jj : 32 * jj + 1, :],
                        colorst[0:MG, t : t + 1],
                        w[:],
                        start=True,
                        stop=True,
                        tile_position=(0, 32 * jj),
                    )
                    if jj == 3:
                        nc.vector.tensor_copy(
                            out_sb[:, b * F : (b + 1) * F], psi_banks[b][:]
                        )

            # consume lags 2 groups behind so img matmuls (which wait on DVE
            # multiplies) never block the tri-matmul stream in the PE queue
            pending = []
            for g in range(8):
                ps = pspool.tile([MG, 2 * F], f32, name="psq")
                for j in range(2):
                    t = 2 * g + j
                    nc.tensor.matmul(
                        ps[:, j * F : (j + 1) * F],
                        tri,
                        lom_grps[t // 4][:, (t % 4) * F : (t % 4 + 1) * F],
                        start=True,
                        stop=True,
                    )
                Tt = tpool.tile([MG, 2 * F], f32, name="Tt", tag="Tt")
                nc.scalar.activation(Tt[:], ps[:], AF.Exp, bias=0.0)
                pending.append((g, Tt))
                if len(pending) > 2:
                    consume(*pending.pop(0))
            for item in pending:
                consume(*item)

            for j in range(4):
                nc.sync.dma_start(out_d[j : j + 1, :], out_sb[32 * j : 32 * j + 1, :])
    nc.compile()
    return nc


_NC = None
LAST_RESULT = None


def _get_nc():
    global _NC
    if _NC is None:
        _NC = _build_nc()
    return _NC


def _prep_inputs(means, quats, scales, rgbs, opacities):
    """Host-side projection + per-tile culling; returns per-core input maps."""
    import ml_dtypes

    means = np.asarray(means, np.float64)
    quats = np.asarray(quats, np.float64)
    scales = np.asarray(scales, np.float64)
    rgbs = np.asarray(rgbs, np.float64)
    opacities = np.asarray(opacities, np.float64)

    c = np.cos(quats)
    s = np.sin(quats)
    sx2 = scales[:, 0] ** 2
    sy2 = scales[:, 1] ** 2
    a11 = c * c * sx2 + s * s * sy2
    a12 = c * s * (sx2 - sy2)
    a22 = s * s * sx2 + c * c * sy2
    det = a11 * a22 - a12 * a12
    ia = a22 / det
    ib = -a12 / det
    ic = a11 / det

    logopac = -np.logaddexp(0.0, -opacities)        # log(sigmoid(o))
    colors = 1.0 / (1.0 + np.exp(-rgbs[:, 0]))      # sigmoid, C=1

    rx = SIGMA_K * np.sqrt(a11)
    ry = SIGMA_K * np.sqrt(a22)
    x0g, x1g = means[:, 0] - rx, means[:, 0] + rx
    y0g, y1g = means[:, 1] - ry, means[:, 1] + ry

    tri = np.triu(np.ones((MG, MG), np.float32), 1)  # lhsT[j,i]=1 iff j<i

    # basis in tile-local coords (identical for every tile)
    fx = (np.arange(F) % TILE_W).astype(np.float64) - (TILE_W - 1) / 2.0
    fy = (np.arange(F) // TILE_W).astype(np.float64) - (TILE_H - 1) / 2.0
    basis5 = np.stack([fx * fx, fx * fy, fy * fy, fx, fy]).astype(np.float32)
    bhi, blo = _split_fp32r(basis5)
    basis = np.concatenate(
        [bhi, blo, bhi, blo, np.ones((2, F), np.float32)]
    )  # [KQ, F]

    in_maps = []
    for core in range(N_CORES):
        qb = np.zeros((QB_H, QB_W), np.float32)
        qb[:, C_BASIS : C_BASIS + F] = basis
        colarr = np.zeros((128, NT), ml_dtypes.bfloat16)
        for t in range(NT):
            tyl, tx = divmod(t, NTX)
            ty = core * 2 + tyl
            X0, X1 = tx * TILE_W, (tx + 1) * TILE_W
            Y0, Y1 = ty * TILE_H, (ty + 1) * TILE_H
            idx = np.nonzero((x1g >= X0) & (x0g <= X1) & (y1g >= Y0) & (y0g <= Y1))[0]
            if len(idx) > MG:
                for k in (3.25, 3.0, 2.75, 2.5, 2.25, 2.0):
                    fac = k / SIGMA_K
                    m = (
                        (means[idx, 0] + fac * rx[idx] >= X0)
                        & (means[idx, 0] - fac * rx[idx] <= X1)
                        & (means[idx, 1] + fac * ry[idx] >= Y0)
                        & (means[idx, 1] - fac * ry[idx] <= Y1)
                    )
                    if m.sum() <= MG:
                        idx = idx[m]
                        break
                else:
                    idx = idx[:MG]
            k = len(idx)
            cx = X0 + (TILE_W - 1) / 2.0 + 0.5   # center of pixel-center range
            cy = Y0 + (TILE_H - 1) / 2.0 + 0.5
            mx = means[idx, 0] - cx
            my = means[idx, 1] - cy
            iag, ibg, icg = ia[idx], ib[idx], ic[idx]
            coef5 = np.zeros((NB, 128), np.float32)
            coef5[0, :k] = -0.5 * iag
            coef5[1, :k] = -ibg
            coef5[2, :k] = -0.5 * icg
            coef5[3, :k] = iag * mx + ibg * my
            coef5[4, :k] = ibg * mx + icg * my
            bias = np.full(128, PAD_BIAS, np.float32)
            bias[:k] = (
                logopac[idx]
                - 0.5 * iag * mx * mx - ibg * mx * my - 0.5 * icg * my * my
            )
            chi, clo = _split_fp32r(coef5)
            bias_hi, bias_lo = _split_fp32r(bias)
            qb[:, C_COEF + t * 128 : C_COEF + (t + 1) * 128] = (
                np.concatenate([chi, chi, clo, clo, bias_hi[None], bias_lo[None]])
            )
            colarr[:k, t] = colors[idx].astype(ml_dtypes.bfloat16)
        in_maps.append({"qb": qb, "tri": tri.copy(), "colors": colarr})
    return in_maps


def kernel(means, quats, scales, rgbs, opacities):
    global LAST_RESULT
    from concourse.bass_utils import run_bass_kernel_spmd

    in_maps = _prep_inputs(means, quats, scales, rgbs, opacities)
    nc = _get_nc()
    res = run_bass_kernel_spmd(nc, in_maps, list(range(N_CORES)))
    LAST_RESULT = res

    img = np.zeros((H, W), np.float32)
    for core in range(N_CORES):
        out = np.asarray(res.results[core]["out"]).reshape(4, 4, TILE_H, TILE_W)
        for t in range(NT):
            b, j = divmod(t, 4)
            tyl, tx = divmod(t, NTX)
            ty = core * 2 + tyl
            img[
                ty * TILE_H : (ty + 1) * TILE_H, tx * TILE_W : (tx + 1) * TILE_W
            ] = out[j, b]
    return img[None, None].astype(np.float32)
